# revision 16
# baseline (speedup 1.0000x reference)
"""Trainium2 Bass kernel for nn_Message_gcn (2-layer RGCN + attention HypergraphConv + info-exchange MLP).

Sharding: pure data parallelism - batch 32 split as 4 samples on each of 8 NeuronCores,
per-layer weights replicated on every core.

v2 layout (PE-continuity focused):
  - graph preprocessing is host-folded: typed adjacency (punct&!aug / aug) is built,
    degree-normalized and shipped as bf16; the hypergraph incidence ships as an additive
    softmax mask (0 / -50), 0.25/D(v) and 1/|e| ship as ready-made columns; layer-0
    node logits a_n and both layers' hyperedge logits a_e (constant across layers)
    are tiny host einsums.
  - per layer, all alpha-independent matmuls (x @ [w_lin | w_rel | w_root]) are emitted
    as one dense per-sample "bulk" block so the PE stays busy (HAM stays at 8/8) while
    DVE/GpSimd/Act run the masked-softmax chains.
  - layer-0 outputs are produced directly in transposed [C, N] layout (the layout layer 1
    consumes): hypergraph out = msg^T @ alpha2^T and RGCN agg = xw^T @ Anorm, with the
    root term applied as an identity-matmul and the bias folded into the PSUM->SBUF
    relu (per-partition Act bias / DVE tensor_scalar).
  - layer-1 outputs are row-major bf16 and stream straight to DRAM; node-0 rows come
    from the info-exchange MLP output.
"""

import sys

sys.path.insert(0, "/opt/trn_rl_repo")

from contextlib import ExitStack

import numpy as np
import ml_dtypes

import concourse.bass as bass
import concourse.tile as tile
from concourse import bacc, mybir
from concourse.bass_utils import run_bass_kernel_spmd

BS, N, E, C, HH, L = 32, 256, 64, 512, 4, 2
M = E + 1
NCORES = 8
BSL = BS // NCORES          # samples per core
NB = N // 128               # node partition tiles
CT = C // 128               # channel partition tiles
C2 = 2 * C
KT2 = C2 // 128             # 2C partition tiles (ie)

f32 = mybir.dt.float32
bf16 = mybir.dt.bfloat16
AF = mybir.ActivationFunctionType
ALU = mybir.AluOpType
AX = mybir.AxisListType


def _ins0(sl: bass.AP, count: int, pos: int) -> bass.AP:
    """Insert a 0-stride (broadcast) dim of `count` into an AP's free dims at
    position `pos` (0 = right after the partition dim, -1 = innermost)."""
    ap = [list(p) for p in sl.ap]
    if pos == -1:
        pos = len(ap) - 1
    ap.insert(1 + pos, [0, count])
    return bass.AP(tensor=sl.tensor, offset=sl.offset, ap=ap)


def _bcast_p(sl: bass.AP, count: int) -> bass.AP:
    """Prepend a 0-stride partition dim of `count` to a DRAM AP (DMA-replicates
    the same source bytes into every partition)."""
    ap = [list(p) for p in sl.ap]
    return bass.AP(tensor=sl.tensor, offset=sl.offset, ap=[[0, count]] + ap)


def build_module():
    nc = bacc.Bacc("TRN2", target_bir_lowering=False, debug=False)

    # ---- DRAM I/O ----
    # all large inputs ship partition-major from the host so every DMA
    # descriptor is >=2KB contiguous per partition
    d_x0T = nc.dram_tensor("x0T", [BSL, 128, CT, N], bf16, kind="ExternalInput")
    d_adjn = nc.dram_tensor("adjn", [BSL, 128, 2, NB, N], bf16, kind="ExternalInput")
    d_hb = nc.dram_tensor("hbm", [BSL, 128, NB, M], bf16, kind="ExternalInput")
    d_ivdq = nc.dram_tensor("ivdq", [BSL, 128, NB], f32, kind="ExternalInput")
    d_ivb = nc.dram_tensor("ivb", [BSL, M, 1], f32, kind="ExternalInput")
    d_an0 = nc.dram_tensor("an0", [BSL, 128, NB, HH], f32, kind="ExternalInput")
    d_ae = nc.dram_tensor("ae", [L, BSL, HH, M], bf16, kind="ExternalInput")
    d_w = nc.dram_tensor("wcomb", [L, 128, 7, CT, 512], bf16, kind="ExternalInput")
    d_ux1 = nc.dram_tensor("ux1", [128, CT, HH], bf16, kind="ExternalInput")
    d_iw1 = nc.dram_tensor("iw1", [L, 128, KT2, C2], bf16, kind="ExternalInput")
    d_iw2 = nc.dram_tensor("iw2", [L, 128, KT2, C2], bf16, kind="ExternalInput")
    d_bcolr = nc.dram_tensor("bcolr", [L, 128, CT], f32, kind="ExternalInput")
    d_bcolh = nc.dram_tensor("bcolh", [L, 128, CT], f32, kind="ExternalInput")
    d_browr = nc.dram_tensor("browr", [L, C], bf16, kind="ExternalInput")
    d_browh = nc.dram_tensor("browh", [L, C], bf16, kind="ExternalInput")
    d_ib1 = nc.dram_tensor("ib1", [L, C2], bf16, kind="ExternalInput")
    d_ib2 = nc.dram_tensor("ib2", [L, C2], bf16, kind="ExternalInput")
    d_eyeb = nc.dram_tensor("eyeb", [128, 128], bf16, kind="ExternalInput")
    d_ones = nc.dram_tensor("onesr", [1, 128], bf16, kind="ExternalInput")
    d_ones4 = nc.dram_tensor("ones4", [1, 4], bf16, kind="ExternalInput")
    d_outr = nc.dram_tensor("outr", [BSL, N, C], bf16, kind="ExternalOutput")
    d_outh = nc.dram_tensor("outh", [BSL, N, C], bf16, kind="ExternalOutput")

    with ExitStack() as ctx:
        tc = ctx.enter_context(tile.TileContext(nc))
        const = ctx.enter_context(tc.tile_pool(name="const", bufs=1))
        wts = ctx.enter_context(tc.tile_pool(name="wts", bufs=2))
        wie = ctx.enter_context(tc.tile_pool(name="wie", bufs=1))
        xst = ctx.enter_context(tc.tile_pool(name="xst", bufs=8))
        acts = ctx.enter_context(tc.tile_pool(name="acts", bufs=3))
        alph = ctx.enter_context(tc.tile_pool(name="alph", bufs=3))
        msgs = ctx.enter_context(tc.tile_pool(name="msgs", bufs=2))
        outs = ctx.enter_context(tc.tile_pool(name="outs", bufs=2))
        iep = ctx.enter_context(tc.tile_pool(name="iep", bufs=1))
        blk = ctx.enter_context(tc.tile_pool(name="blk", bufs=4, space="PSUM"))
        pst = ctx.enter_context(tc.tile_pool(name="pst", bufs=2, space="PSUM"))
        pmx = ctx.enter_context(tc.tile_pool(name="pmx", bufs=2, space="PSUM"))

        # round-robin over the two PSUM-capable copy engines
        _eng = [0]

        def cpeng():
            _eng[0] ^= 1
            return nc.vector if _eng[0] else nc.scalar

        def relu_bias(dst, src, bias_ap):
            e = cpeng()
            if e is nc.scalar:
                e.activation(dst, src, AF.Relu, bias=bias_ap)
            else:
                e.tensor_scalar(dst, src, bias_ap, 0.0, op0=ALU.add, op1=ALU.max)

        def copy(dst, src, scale=None):
            e = cpeng()
            if e is nc.scalar:
                if scale is None:
                    e.copy(dst, src)
                else:
                    e.activation(dst, src, AF.Copy, scale=scale)
            else:
                if scale is None:
                    e.tensor_copy(dst, src)
                else:
                    e.tensor_scalar(dst, src, scale, None, op0=ALU.mult)

        # ---- input / constant DMAs ----
        # sync queue carries the bulk-critical stream in consumption order:
        # sample-0 inputs + first weight chunks first, so the PE can start
        # within a few us of kernel start.
        x0Ts = [None] * BSL
        ws = [None] * L
        for s in range(BSL):
            x0Ts[s] = xst.tile([128, CT, N], bf16, tag="xst", name=f"x0T_{s}")
        ws[0] = wts.tile([128, 7, CT, 512], bf16, tag="w", name="w0")
        nc.sync.dma_start(x0Ts[0][:], d_x0T[0])
        for k in range(7):
            nc.sync.dma_start(ws[0][:, k, :, :], d_w[0, :, k, :, :])
            if k < 3:
                nc.sync.dma_start(x0Ts[k + 1][:], d_x0T[k + 1])

        identb = const.tile([128, 128], bf16)
        nc.sync.dma_start(identb[:], d_eyeb[:])
        ones_row = const.tile([1, 128], bf16)
        nc.sync.dma_start(ones_row[:], d_ones[:])
        ones4 = const.tile([1, 4], bf16)
        nc.sync.dma_start(ones4[:], d_ones4[:])
        ux1 = const.tile([128, CT, HH], bf16)
        nc.sync.dma_start(ux1[:], d_ux1[:])

        an0s, abs_, hbs, ivdqs, ivbs, Afns = [], [[], []], [], [], [], []
        for s in range(BSL):
            t = const.tile([128, NB, HH], f32, name=f"an0_{s}")
            nc.scalar.dma_start(t[:], d_an0[s])
            an0s.append(t)
            for l in range(L):
                a = const.tile([128, HH, M], bf16, name=f"ab{l}_{s}")
                nc.scalar.dma_start(a[:], _bcast_p(d_ae[l, s], 128))
                abs_[l].append(a)
            t = const.tile([128, NB, M], bf16, name=f"hb_{s}")
            nc.scalar.dma_start(t[:], d_hb[s])
            hbs.append(t)
            t = const.tile([128, NB], f32, name=f"ivdq_{s}")
            nc.scalar.dma_start(t[:], d_ivdq[s])
            ivdqs.append(t)
            t = const.tile([M, 1], f32, name=f"ivb_{s}")
            nc.scalar.dma_start(t[:], d_ivb[s])
            ivbs.append(t)
            t = const.tile([128, 2, NB, N], bf16, name=f"afn_{s}")
            nc.gpsimd.dma_start(t[:], d_adjn[s])
            Afns.append(t)

        bcolr, bcolh, browr, browh, ib1r, ib2r = [], [], [], [], [], []
        for l in range(L):
            t = const.tile([128, CT], f32, name=f"bcolr{l}")
            nc.scalar.dma_start(t[:], d_bcolr[l])
            bcolr.append(t)
            t = const.tile([128, CT], f32, name=f"bcolh{l}")
            nc.scalar.dma_start(t[:], d_bcolh[l])
            bcolh.append(t)
            t = const.tile([1, C], bf16, name=f"browr{l}")
            nc.scalar.dma_start(t[:], d_browr[l : l + 1, :])
            browr.append(t)
            t = const.tile([1, C], bf16, name=f"browh{l}")
            nc.scalar.dma_start(t[:], d_browh[l : l + 1, :])
            browh.append(t)
            t = const.tile([1, C2], bf16, name=f"ib1_{l}")
            nc.scalar.dma_start(t[:], d_ib1[l : l + 1, :])
            ib1r.append(t)
            t = const.tile([1, C2], bf16, name=f"ib2_{l}")
            nc.scalar.dma_start(t[:], d_ib2[l : l + 1, :])
            ib2r.append(t)


        # ---- per-(layer, sample) pieces ----
        xrTs = list(x0Ts)
        xhTs = list(x0Ts)
        alphas = [None] * BSL
        a2bs = [None] * BSL
        xls = [None] * BSL
        xws = [None] * BSL
        xwrs = [None] * BSL
        an1s = [None] * BSL
        ctxT = None

        def alpha_chain(l, s):
            """Masked softmax over incident hyperedges -> alpha (for msg) and
            a2b = alpha * 0.25/D(v) (for the node-side aggregation).
            GpSimd builds the logits, Act does the exp, DVE the reductions."""
            an_src = an0s[s] if l == 0 else an1s[s]
            t1 = alph.tile([128, NB, HH, M], f32, tag="t1", bufs=2)
            nc.gpsimd.tensor_tensor(t1[:], _ins0(abs_[l][s][:], NB, 0),
                                    _ins0(an_src[:], M, -1), op=ALU.add)
            nc.vector.scalar_tensor_tensor(t1[:], t1[:], 0.2, t1[:],
                                           op0=ALU.mult, op1=ALU.max)
            nc.gpsimd.tensor_tensor(t1[:], t1[:], _ins0(hbs[s][:], HH, 1), op=ALU.add)
            nmax = alph.tile([128, NB, HH], f32, tag="nmax")
            nc.vector.tensor_reduce(nmax[:], t1[:], axis=AX.X, op=ALU.max, negate=True)
            nc.gpsimd.tensor_tensor(t1[:], t1[:], _ins0(nmax[:], M, -1), op=ALU.add)
            nc.scalar.activation(t1[:], t1[:], AF.Exp)
            ssum = alph.tile([128, NB, HH], f32, tag="ssum")
            nc.vector.tensor_reduce(ssum[:], t1[:], axis=AX.X, op=ALU.add)
            rs = alph.tile([128, NB, HH], f32, tag="rs")
            nc.vector.reciprocal(rs[:], ssum[:])
            rcol2 = alph.tile([128, NB, HH], f32, tag="rcol2")
            nc.vector.tensor_tensor(rcol2[:], rs[:], _ins0(ivdqs[s][:], HH, -1), op=ALU.mult)
            al = alph.tile([128, NB, HH, M], bf16, tag="alpha")
            nc.vector.tensor_tensor(al[:], t1[:], _ins0(rs[:], M, -1), op=ALU.mult)
            a2 = alph.tile([128, NB, HH, M], bf16, tag="a2b")
            nc.vector.tensor_tensor(a2[:], t1[:], _ins0(rcol2[:], M, -1), op=ALU.mult)
            alphas[s] = al
            a2bs[s] = a2

        def bulk(l, s):
            """x @ [w_lin | w_rel0 | w_rel1 | w_root]: all alpha-independent
            PE work for (l, s), drained chunk-by-chunk into bf16 SBUF."""
            xl = acts.tile([128, HH, NB, C], bf16, tag="xl")
            xw = acts.tile([128, NB, 2, C], bf16, tag="xw")
            xwr = acts.tile([128, NB, C], bf16, tag="xwr")
            for k in range(7):
                for nt in range(NB):
                    if k < HH:
                        stat = xhTs[s]
                        dst = xl[:, k, nt, :]
                    else:
                        stat = xrTs[s]
                        dst = xw[:, nt, k - 4, :] if k < 6 else xwr[:, nt, :]
                    ps = blk.tile([128, C], f32, tag="blk")
                    for ct in range(CT):
                        nc.tensor.matmul(ps[:], stat[:, ct, nt * 128 : (nt + 1) * 128],
                                         ws[l][:, k, ct, :], start=(ct == 0), stop=(ct == CT - 1))
                    copy(dst, ps[:])
            xls[s], xws[s], xwrs[s] = xl, xw, xwr

        def phase2(l, s):
            """alpha-dependent per-sample work: alpha transposes, msg,
            hypergraph out, RGCN aggregation, ctx column."""
            al, a2 = alphas[s], a2bs[s]
            xl, xw, xwr = xls[s], xws[s], xwrs[s]
            Afn = Afns[s]

            a3T = msgs.tile([M, HH, N], bf16, tag="a3T")
            for nb in range(NB):
                for h in range(HH):
                    tp = pmx.tile([M, 128], bf16, tag="mx", padded_shape=[M, 1024])
                    nc.tensor.transpose(tp[:], a2[:, nb, h, :], identb[:])
                    copy(a3T[:, h, nb * 128 : (nb + 1) * 128], tp[:])

            msg = msgs.tile([M, HH, C], bf16, tag="msg")
            for h in range(HH):
                mp = blk.tile([M, C], f32, tag="blk")
                for nb in range(NB):
                    nc.tensor.matmul(mp[:], al[:, nb, h, :], xl[:, h, nb, :],
                                     start=(nb == 0), stop=(nb == NB - 1))
                copy(msg[:, h, :], mp[:], scale=ivbs[s][:, 0:1])

            if l == 0:
                # transposed outputs -> next-layer state tiles [C, N]
                xhT1 = xst.tile([128, CT, N], bf16, tag="xst")
                xrT1 = xst.tile([128, CT, N], bf16, tag="xst")
                for g in range(CT // 2):
                    ph = pst.tile([128, 2, N], f32, tag="st")
                    for j in range(2):
                        ct = 2 * g + j
                        for h in range(HH):
                            nc.tensor.matmul(ph[:, j, :], msg[:, h, ct * 128 : (ct + 1) * 128],
                                             a3T[:, h, :], start=(h == 0), stop=(h == HH - 1))
                    for j in range(2):
                        ct = 2 * g + j
                        relu_bias(xhT1[:, ct, :], ph[:, j, :], bcolh[l][:, ct : ct + 1])
                for g in range(CT // 2):
                    pr = pst.tile([128, 2, N], f32, tag="st")
                    for j in range(2):
                        dt = 2 * g + j
                        first = True
                        for r in range(2):
                            for it in range(NB):
                                nc.tensor.matmul(pr[:, j, :],
                                                 xw[:, it, r, dt * 128 : (dt + 1) * 128],
                                                 Afn[:, r, it, :], start=first, stop=False)
                                first = False
                        for it in range(NB):
                            nc.tensor.matmul(pr[:, j, it * 128 : (it + 1) * 128],
                                             xwr[:, it, dt * 128 : (dt + 1) * 128],
                                             identb[:], start=False, stop=(it == NB - 1))
                    for j in range(2):
                        dt = 2 * g + j
                        relu_bias(xrT1[:, dt, :], pr[:, j, :], bcolr[l][:, dt : dt + 1])
                xhTs[s] = xhT1
                xrTs[s] = xrT1
                # ctx columns (pre-patch node-0 output)
                nc.vector.tensor_copy(ctxT[:, 0:CT, s : s + 1], xrT1[:, :, 0:1])
                nc.vector.tensor_copy(ctxT[:, CT : 2 * CT, s : s + 1], xhT1[:, :, 0:1])
            else:
                # row-major bf16 outputs, streamed to DRAM (rows 1..N-1)
                outh_t = outs.tile([128, NB, C], bf16, tag="outh")
                for nb in range(NB):
                    po = blk.tile([128, C], f32, tag="blk")
                    for h in range(HH):
                        nc.tensor.matmul(po[:], a3T[:, h, nb * 128 : (nb + 1) * 128],
                                         msg[:, h, :], start=(h == 0), stop=False)
                    nc.tensor.matmul(po[:], ones_row[:], browh[l][:], start=False, stop=True)
                    e = cpeng()
                    if e is nc.scalar:
                        e.activation(outh_t[:, nb, :], po[:], AF.Relu)
                    else:
                        e.tensor_scalar(outh_t[:, nb, :], po[:], 0.0, None, op0=ALU.max)
                outr_t = outs.tile([128, NB, C], bf16, tag="outr")
                for jb in range(NB):
                    po = blk.tile([128, C], f32, tag="blk")
                    first = True
                    for r in range(2):
                        for it in range(NB):
                            nc.tensor.matmul(po[:], Afn[:, r, it, jb * 128 : (jb + 1) * 128],
                                             xw[:, it, r, :], start=first, stop=False)
                            first = False
                    nc.tensor.matmul(po[:], identb[:], xwr[:, jb, :], start=False, stop=False)
                    nc.tensor.matmul(po[:], ones_row[:], browr[l][:], start=False, stop=True)
                    e = cpeng()
                    if e is nc.scalar:
                        e.activation(outr_t[:, jb, :], po[:], AF.Relu)
                    else:
                        e.tensor_scalar(outr_t[:, jb, :], po[:], 0.0, None, op0=ALU.max)
                # ctx rows -> columns
                cps = pmx.tile([128, 2, CT, 2], bf16, tag="mx", padded_shape=[128, 2, CT, 64])
                for ct in range(CT):
                    nc.tensor.transpose(cps[:, 0, ct, 0:1],
                                        outr_t[0:1, 0, ct * 128 : (ct + 1) * 128],
                                        identb[0:1, 0:1])
                    nc.tensor.transpose(cps[:, 1, ct, 0:1],
                                        outh_t[0:1, 0, ct * 128 : (ct + 1) * 128],
                                        identb[0:1, 0:1])
                nc.vector.tensor_copy(
                    ctxT[:, :, s : s + 1].rearrange("p (r ct) o -> p r (ct o)", r=2),
                    cps[:, :, :, 0])
                for tsrc, dram in ((outr_t, d_outr), (outh_t, d_outh)):
                    nc.sync.dma_start(dram[s, 1:128, :], tsrc[1:128, 0, :])
                    nc.sync.dma_start(dram[s, 128:N, :], tsrc[:, 1, :])

        def ie(l):
            """info-exchange MLP over the BSL ctx columns; layer 0 patches the
            state tiles' node-0 column, layer 1 DMAs the node-0 output rows."""
            y1 = iep.tile([BSL, C2], bf16, tag="y1")
            for ch in range(2):
                yp = blk.tile([BSL, C], f32, tag="blk")
                for kt in range(KT2):
                    nc.tensor.matmul(yp[:], ctxT[:, kt, :], iw1_t[:, kt, ch * C : (ch + 1) * C],
                                     start=(kt == 0), stop=False)
                nc.tensor.matmul(yp[:], ones4[:], ib1r[l][:, ch * C : (ch + 1) * C],
                                 start=False, stop=True)
                e = cpeng()
                if e is nc.scalar:
                    e.activation(y1[:, ch * C : (ch + 1) * C], yp[:], AF.Relu)
                else:
                    e.tensor_scalar(y1[:, ch * C : (ch + 1) * C], yp[:], 0.0, None, op0=ALU.max)
            c2T = iep.tile([128, KT2, BSL], bf16, tag="c2T")
            for kt in range(KT2):
                tp = pmx.tile([128, BSL], bf16, tag="mx", padded_shape=[128, 1024])
                nc.tensor.transpose(tp[:], y1[:, kt * 128 : (kt + 1) * 128], identb[0:BSL, 0:BSL])
                copy(c2T[:, kt, :], tp[:])
            y2 = iep.tile([BSL, C2], bf16, tag="y2")
            for ch in range(2):
                yp = blk.tile([BSL, C], f32, tag="blk")
                for kt in range(KT2):
                    nc.tensor.matmul(yp[:], c2T[:, kt, :], iw2_t[:, kt, ch * C : (ch + 1) * C],
                                     start=(kt == 0), stop=False)
                nc.tensor.matmul(yp[:], ones4[:], ib2r[l][:, ch * C : (ch + 1) * C],
                                 start=False, stop=True)
                copy(y2[:, ch * C : (ch + 1) * C], yp[:])
            if l == 0:
                for kt in range(KT2):
                    tp = pmx.tile([128, BSL], bf16, tag="mx", padded_shape=[128, 1024])
                    nc.tensor.transpose(tp[:], y2[:, kt * 128 : (kt + 1) * 128],
                                        identb[0:BSL, 0:BSL])
                    for s in range(BSL):
                        dst = xrTs[s] if kt < CT else xhTs[s]
                        e = cpeng()
                        if e is nc.scalar:
                            e.copy(dst[:, kt % CT, 0:1], tp[:, s : s + 1])
                        else:
                            e.tensor_copy(dst[:, kt % CT, 0:1], tp[:, s : s + 1])
            else:
                for s in range(BSL):
                    nc.sync.dma_start(d_outr[s, 0:1, :], y2[s : s + 1, 0:C])
                    nc.sync.dma_start(d_outh[s, 0:1, :], y2[s : s + 1, C:C2])

        # ================= layer 0 =================
        ctxT = iep.tile([128, 2 * CT, BSL], bf16, tag="ctxT")
        for s in range(BSL):
            alpha_chain(0, s)
        # layer-1 / IE weights stream behind the critical path: their issue
        # instructions sit after the alpha-chain work on the gpsimd/vector
        # queues, so the transfers start only once layer 0 is underway
        ws[1] = wts.tile([128, 7, CT, 512], bf16, tag="w", name="w1")
        for k in range(7):
            nc.gpsimd.dma_start(ws[1][:, k, :, :], d_w[1, :, k, :, :])
        iw1_t = wie.tile([128, KT2, C2], bf16, tag="iw1")
        nc.scalar.dma_start(iw1_t[:], d_iw1[0])
        iw2_t = wie.tile([128, KT2, C2], bf16, tag="iw2")
        nc.scalar.dma_start(iw2_t[:], d_iw2[0])
        bulk(0, 0)
        bulk(0, 1)
        phase2(0, 0)
        bulk(0, 2)
        phase2(0, 1)
        bulk(0, 3)
        phase2(0, 2)
        phase2(0, 3)
        ie(0)

        # layer-1 IE weights reuse the same SBUF slots (gated on ie(0) readers)
        iw1_t = wie.tile([128, KT2, C2], bf16, tag="iw1")
        nc.gpsimd.dma_start(iw1_t[:], d_iw1[1])
        iw2_t = wie.tile([128, KT2, C2], bf16, tag="iw2")
        nc.sync.dma_start(iw2_t[:], d_iw2[1])

        # ================= layer 1 =================
        ctxT = iep.tile([128, 2 * CT, BSL], bf16, tag="ctxT2")
        for s in range(BSL):
            ap = blk.tile([HH, N], f32, tag="blk", padded_shape=[HH, 512])
            for ct in range(CT):
                nc.tensor.matmul(ap[:], ux1[:, ct, :], xhTs[s][:, ct, :],
                                 start=(ct == 0), stop=(ct == CT - 1))
            anrow = iep.tile([HH, N], bf16, tag="anrow", name=f"anrow_{s}")
            copy(anrow[:], ap[:])
            an1 = alph.tile([128, NB, HH], f32, tag="an1")
            for nb in range(NB):
                tp = pmx.tile([128, HH], bf16, tag="mx", padded_shape=[128, 1024])
                nc.tensor.transpose(tp[:], anrow[:, nb * 128 : (nb + 1) * 128],
                                    identb[0:HH, 0:HH])
                copy(an1[:, nb, :], tp[:])
            an1s[s] = an1
        for s in range(BSL):
            alpha_chain(1, s)
        bulk(1, 0)
        bulk(1, 1)
        phase2(1, 0)
        bulk(1, 2)
        phase2(1, 1)
        bulk(1, 3)
        phase2(1, 2)
        phase2(1, 3)
        ie(1)

    nc.compile()
    return nc


_NC = None


def _get_nc():
    global _NC
    if _NC is None:
        _NC = build_module()
    return _NC


def make_in_maps(encoded_spans, SVO_emb, pooled_output, sent2word_adj, aug_adj,
                 punct_graph, w_rel, w_root, b_rgcn, w_lin, att_x, att_e, b_hgcn,
                 ie_w1, ie_b1, ie_w2, ie_b2):
    f = np.float32
    bf = ml_dtypes.bfloat16
    x = np.asarray(encoded_spans, f)                               # [BS, N, C]
    aug = np.asarray(aug_adj, f)
    pun = np.asarray(punct_graph, f)
    A = np.stack([pun * (1.0 - aug), aug], axis=1)                 # [BS, 2, N, N]
    deg = A.sum(axis=2)                                            # in-degree of target j
    adjn = A / np.where(deg > 0, deg, 1.0)[:, :, None, :]
    Hinc = np.concatenate([np.ones((BS, N, 1), f),
                           np.asarray(sent2word_adj, f)], axis=2)  # [BS, N, M]
    hbm = np.where(Hinc > 0, 0.0, -50.0).astype(f)
    Dn = Hinc.sum(axis=2)
    ivdq = (0.25 / np.where(Dn > 0, Dn, 1.0)).astype(f)            # [BS, N]
    Be = Hinc.sum(axis=1)
    ivb = np.where(Be > 0, 1.0 / np.where(Be > 0, Be, 1.0), 0.0).astype(f)  # [BS, M]
    e_attr = np.concatenate([np.asarray(pooled_output, f)[:, None, :],
                             np.asarray(SVO_emb, f)], axis=1)      # [BS, M, C]
    wl = np.ascontiguousarray(np.asarray(w_lin, f))                # [L, C, HH*C]
    wl4 = wl.reshape(L, C, HH, C)
    ux = np.einsum("lchk,lhk->lch", wl4, np.asarray(att_x, f))     # [L, C, HH]
    ue = np.einsum("lchk,lhk->lch", wl4, np.asarray(att_e, f))
    an0 = np.einsum("bnc,ch->bnh", x, ux[0]).astype(f)             # [BS, N, HH]
    ae = np.einsum("bmc,lch->lbhm", e_attr, ue)                    # [L, BS, HH, M]
    wr = np.asarray(w_rel, f)
    wcat = np.concatenate([wr[:, 0], wr[:, 1], np.asarray(w_root, f)], axis=2)
    x0T = np.ascontiguousarray(x.transpose(0, 2, 1))               # [BS, C, N]

    # partition-major swizzles: index (ct*128+p) -> [p][ct]
    wcomb = np.concatenate([wl, wcat], axis=2)                     # [L, C, 3584]
    wcomb = wcomb.reshape(L, CT, 128, 7, 512).transpose(0, 2, 3, 1, 4)  # [L,128,7,CT,512]
    x0Tp = x0T.reshape(BS, CT, 128, N).transpose(0, 2, 1, 3)       # [BS,128,CT,N]
    adjnp = adjn.reshape(BS, 2, NB, 128, N).transpose(0, 3, 1, 2, 4)  # [BS,128,2,NB,N]
    hbp = hbm.reshape(BS, NB, 128, M).transpose(0, 2, 1, 3)        # [BS,128,NB,M]
    an0p = an0.reshape(BS, NB, 128, HH).transpose(0, 2, 1, 3)      # [BS,128,NB,HH]
    ivdqp = ivdq.reshape(BS, NB, 128).transpose(0, 2, 1)           # [BS,128,NB]
    iw1p = np.asarray(ie_w1, f).reshape(L, KT2, 128, C2).transpose(0, 2, 1, 3)
    iw2p = np.asarray(ie_w2, f).reshape(L, KT2, 128, C2).transpose(0, 2, 1, 3)
    ux1p = ux[1].reshape(CT, 128, HH).transpose(1, 0, 2)           # [128,CT,HH]
    bcolrp = np.asarray(b_rgcn, f).reshape(L, CT, 128).transpose(0, 2, 1)
    bcolhp = np.asarray(b_hgcn, f).reshape(L, CT, 128).transpose(0, 2, 1)

    shared = {
        "wcomb": np.ascontiguousarray(wcomb).astype(bf),
        "ux1": np.ascontiguousarray(ux1p).astype(bf),
        "iw1": np.ascontiguousarray(iw1p).astype(bf),
        "iw2": np.ascontiguousarray(iw2p).astype(bf),
        "bcolr": np.ascontiguousarray(bcolrp),
        "bcolh": np.ascontiguousarray(bcolhp),
        "browr": np.asarray(b_rgcn, f).astype(bf),
        "browh": np.asarray(b_hgcn, f).astype(bf),
        "ib1": np.asarray(ie_b1, f).astype(bf),
        "ib2": np.asarray(ie_b2, f).astype(bf),
        "eyeb": np.eye(128, dtype=f).astype(bf),
        "onesr": np.ones((1, 128), f).astype(bf),
        "ones4": np.ones((1, 4), f).astype(bf),
    }
    in_maps = []
    for c in range(NCORES):
        sl = slice(c * BSL, (c + 1) * BSL)
        m = dict(shared)
        m["x0T"] = np.ascontiguousarray(x0Tp[sl]).astype(bf)
        m["adjn"] = np.ascontiguousarray(adjnp[sl]).astype(bf)
        m["hbm"] = np.ascontiguousarray(hbp[sl]).astype(bf)
        m["ivdq"] = np.ascontiguousarray(ivdqp[sl])
        m["ivb"] = np.ascontiguousarray(ivb[sl])[:, :, None]
        m["an0"] = np.ascontiguousarray(an0p[sl])
        m["ae"] = np.ascontiguousarray(ae[:, sl]).astype(bf)
        in_maps.append(m)
    return in_maps


def run(in_maps, trace=False, **kw):
    nc = _get_nc()
    return run_bass_kernel_spmd(nc, in_maps, list(range(NCORES)), trace=trace, **kw)


def kernel(**inputs):
    in_maps = make_in_maps(**inputs)
    res = run(in_maps)
    x_r = np.concatenate([np.asarray(res.results[c]["outr"]) for c in range(NCORES)], axis=0)
    x_h = np.concatenate([np.asarray(res.results[c]["outh"]) for c in range(NCORES)], axis=0)
    return x_r.astype(np.float32), x_h.astype(np.float32)


# revision 18
# speedup vs baseline: 1.0022x; 1.0022x over previous
"""Trainium2 Bass kernel for nn_Message_gcn (2-layer RGCN + attention HypergraphConv + info-exchange MLP).

Sharding: pure data parallelism - batch 32 split as 4 samples on each of 8 NeuronCores,
per-layer weights replicated on every core.

v2 layout (PE-continuity focused):
  - graph preprocessing is host-folded: typed adjacency (punct&!aug / aug) is built,
    degree-normalized and shipped as bf16; the hypergraph incidence ships as an additive
    softmax mask (0 / -50), 0.25/D(v) and 1/|e| ship as ready-made columns; layer-0
    node logits a_n and both layers' hyperedge logits a_e (constant across layers)
    are tiny host einsums.
  - per layer, all alpha-independent matmuls (x @ [w_lin | w_rel | w_root]) are emitted
    as one dense per-sample "bulk" block so the PE stays busy (HAM stays at 8/8) while
    DVE/GpSimd/Act run the masked-softmax chains.
  - layer-0 outputs are produced directly in transposed [C, N] layout (the layout layer 1
    consumes): hypergraph out = msg^T @ alpha2^T and RGCN agg = xw^T @ Anorm, with the
    root term applied as an identity-matmul and the bias folded into the PSUM->SBUF
    relu (per-partition Act bias / DVE tensor_scalar).
  - layer-1 outputs are row-major bf16 and stream straight to DRAM; node-0 rows come
    from the info-exchange MLP output.
"""

import sys

sys.path.insert(0, "/opt/trn_rl_repo")

from contextlib import ExitStack

import numpy as np
import ml_dtypes

import concourse.bass as bass
import concourse.tile as tile
from concourse import bacc, mybir
from concourse.bass_utils import run_bass_kernel_spmd

BS, N, E, C, HH, L = 32, 256, 64, 512, 4, 2
M = E + 1
NCORES = 8
BSL = BS // NCORES          # samples per core
NB = N // 128               # node partition tiles
CT = C // 128               # channel partition tiles
C2 = 2 * C
KT2 = C2 // 128             # 2C partition tiles (ie)

f32 = mybir.dt.float32
bf16 = mybir.dt.bfloat16
AF = mybir.ActivationFunctionType
ALU = mybir.AluOpType
AX = mybir.AxisListType


def _ins0(sl: bass.AP, count: int, pos: int) -> bass.AP:
    """Insert a 0-stride (broadcast) dim of `count` into an AP's free dims at
    position `pos` (0 = right after the partition dim, -1 = innermost)."""
    ap = [list(p) for p in sl.ap]
    if pos == -1:
        pos = len(ap) - 1
    ap.insert(1 + pos, [0, count])
    return bass.AP(tensor=sl.tensor, offset=sl.offset, ap=ap)


def _bcast_p(sl: bass.AP, count: int) -> bass.AP:
    """Prepend a 0-stride partition dim of `count` to a DRAM AP (DMA-replicates
    the same source bytes into every partition)."""
    ap = [list(p) for p in sl.ap]
    return bass.AP(tensor=sl.tensor, offset=sl.offset, ap=[[0, count]] + ap)


def build_module():
    nc = bacc.Bacc("TRN2", target_bir_lowering=False, debug=False)

    # ---- DRAM I/O ----
    # all large inputs ship partition-major from the host so every DMA
    # descriptor is >=2KB contiguous per partition
    d_x0T = nc.dram_tensor("x0T", [BSL, 128, CT, N], bf16, kind="ExternalInput")
    d_adjn = nc.dram_tensor("adjn", [BSL, 128, 2, NB, N], bf16, kind="ExternalInput")
    d_hb = nc.dram_tensor("hbm", [BSL, 128, NB, M], bf16, kind="ExternalInput")
    d_ivdq = nc.dram_tensor("ivdq", [BSL, 128, NB], f32, kind="ExternalInput")
    d_ivb = nc.dram_tensor("ivb", [BSL, M, 1], f32, kind="ExternalInput")
    d_an0 = nc.dram_tensor("an0", [BSL, 128, NB, HH], f32, kind="ExternalInput")
    d_ae = nc.dram_tensor("ae", [L, BSL, HH, M], bf16, kind="ExternalInput")
    d_w = nc.dram_tensor("wcomb", [L, 128, 7, CT, 512], bf16, kind="ExternalInput")
    d_ux1 = nc.dram_tensor("ux1", [128, CT, HH], bf16, kind="ExternalInput")
    d_iw1 = nc.dram_tensor("iw1", [L, 128, KT2, C2], bf16, kind="ExternalInput")
    d_iw2 = nc.dram_tensor("iw2", [L, 128, KT2, C2], bf16, kind="ExternalInput")
    d_bcolr = nc.dram_tensor("bcolr", [L, 128, CT], f32, kind="ExternalInput")
    d_bcolh = nc.dram_tensor("bcolh", [L, 128, CT], f32, kind="ExternalInput")
    d_browr = nc.dram_tensor("browr", [L, C], bf16, kind="ExternalInput")
    d_browh = nc.dram_tensor("browh", [L, C], bf16, kind="ExternalInput")
    d_ib1 = nc.dram_tensor("ib1", [L, C2], bf16, kind="ExternalInput")
    d_ib2 = nc.dram_tensor("ib2", [L, C2], bf16, kind="ExternalInput")
    d_eyeb = nc.dram_tensor("eyeb", [128, 128], bf16, kind="ExternalInput")
    d_ones = nc.dram_tensor("onesr", [1, 128], bf16, kind="ExternalInput")
    d_ones4 = nc.dram_tensor("ones4", [1, 4], bf16, kind="ExternalInput")
    d_outr = nc.dram_tensor("outr", [BSL, N, C], bf16, kind="ExternalOutput")
    d_outh = nc.dram_tensor("outh", [BSL, N, C], bf16, kind="ExternalOutput")

    with ExitStack() as ctx:
        tc = ctx.enter_context(tile.TileContext(nc))
        const = ctx.enter_context(tc.tile_pool(name="const", bufs=1))
        wts = ctx.enter_context(tc.tile_pool(name="wts", bufs=2))
        wie = ctx.enter_context(tc.tile_pool(name="wie", bufs=1))
        xst = ctx.enter_context(tc.tile_pool(name="xst", bufs=8))
        acts = ctx.enter_context(tc.tile_pool(name="acts", bufs=3))
        alph = ctx.enter_context(tc.tile_pool(name="alph", bufs=3))
        msgs = ctx.enter_context(tc.tile_pool(name="msgs", bufs=2))
        outs = ctx.enter_context(tc.tile_pool(name="outs", bufs=2))
        iep = ctx.enter_context(tc.tile_pool(name="iep", bufs=1))
        blk = ctx.enter_context(tc.tile_pool(name="blk", bufs=4, space="PSUM"))
        pst = ctx.enter_context(tc.tile_pool(name="pst", bufs=2, space="PSUM"))
        pmx = ctx.enter_context(tc.tile_pool(name="pmx", bufs=2, space="PSUM"))

        # round-robin over the two PSUM-capable copy engines
        _eng = [0]

        def cpeng():
            _eng[0] ^= 1
            return nc.vector if _eng[0] else nc.scalar

        def relu_bias(dst, src, bias_ap):
            e = cpeng()
            if e is nc.scalar:
                e.activation(dst, src, AF.Relu, bias=bias_ap)
            else:
                e.tensor_scalar(dst, src, bias_ap, 0.0, op0=ALU.add, op1=ALU.max)

        def copy(dst, src, scale=None):
            e = cpeng()
            if e is nc.scalar:
                if scale is None:
                    e.copy(dst, src)
                else:
                    e.activation(dst, src, AF.Copy, scale=scale)
            else:
                if scale is None:
                    e.tensor_copy(dst, src)
                else:
                    e.tensor_scalar(dst, src, scale, None, op0=ALU.mult)

        # ---- input / constant DMAs ----
        # sync queue carries the bulk-critical stream in consumption order:
        # sample-0 inputs + first weight chunks first, so the PE can start
        # within a few us of kernel start.
        x0Ts = [None] * BSL
        ws = [None] * L
        for s in range(BSL):
            x0Ts[s] = xst.tile([128, CT, N], bf16, tag="xst", name=f"x0T_{s}")
        ws[0] = wts.tile([128, 7, CT, 512], bf16, tag="w", name="w0")
        nc.sync.dma_start(x0Ts[0][:], d_x0T[0])
        for k in range(7):
            nc.sync.dma_start(ws[0][:, k, :, :], d_w[0, :, k, :, :])
            if k < 3:
                nc.sync.dma_start(x0Ts[k + 1][:], d_x0T[k + 1])

        identb = const.tile([128, 128], bf16)
        nc.sync.dma_start(identb[:], d_eyeb[:])
        ones_row = const.tile([1, 128], bf16)
        nc.sync.dma_start(ones_row[:], d_ones[:])
        ones4 = const.tile([1, 4], bf16)
        nc.sync.dma_start(ones4[:], d_ones4[:])
        ux1 = const.tile([128, CT, HH], bf16)
        nc.sync.dma_start(ux1[:], d_ux1[:])

        an0s, abs_, hbs, ivdqs, ivbs, Afns = [], [[], []], [], [], [], []
        for s in range(BSL):
            t = const.tile([128, NB, HH], f32, name=f"an0_{s}")
            nc.scalar.dma_start(t[:], d_an0[s])
            an0s.append(t)
            for l in range(L):
                a = const.tile([128, HH, M], bf16, name=f"ab{l}_{s}")
                nc.scalar.dma_start(a[:], _bcast_p(d_ae[l, s], 128))
                abs_[l].append(a)
            t = const.tile([128, NB, M], bf16, name=f"hb_{s}")
            nc.scalar.dma_start(t[:], d_hb[s])
            hbs.append(t)
            t = const.tile([128, NB], f32, name=f"ivdq_{s}")
            nc.scalar.dma_start(t[:], d_ivdq[s])
            ivdqs.append(t)
            t = const.tile([M, 1], f32, name=f"ivb_{s}")
            nc.scalar.dma_start(t[:], d_ivb[s])
            ivbs.append(t)
            t = const.tile([128, 2, NB, N], bf16, name=f"afn_{s}")
            nc.gpsimd.dma_start(t[:], d_adjn[s])
            Afns.append(t)

        bcolr, bcolh, browr, browh, ib1r, ib2r = [], [], [], [], [], []
        for l in range(L):
            t = const.tile([128, CT], f32, name=f"bcolr{l}")
            nc.scalar.dma_start(t[:], d_bcolr[l])
            bcolr.append(t)
            t = const.tile([128, CT], f32, name=f"bcolh{l}")
            nc.scalar.dma_start(t[:], d_bcolh[l])
            bcolh.append(t)
            t = const.tile([1, C], bf16, name=f"browr{l}")
            nc.scalar.dma_start(t[:], d_browr[l : l + 1, :])
            browr.append(t)
            t = const.tile([1, C], bf16, name=f"browh{l}")
            nc.scalar.dma_start(t[:], d_browh[l : l + 1, :])
            browh.append(t)
            t = const.tile([1, C2], bf16, name=f"ib1_{l}")
            nc.scalar.dma_start(t[:], d_ib1[l : l + 1, :])
            ib1r.append(t)
            t = const.tile([1, C2], bf16, name=f"ib2_{l}")
            nc.scalar.dma_start(t[:], d_ib2[l : l + 1, :])
            ib2r.append(t)


        # ---- per-(layer, sample) pieces ----
        xrTs = list(x0Ts)
        xhTs = list(x0Ts)
        alphas = [None] * BSL
        a2bs = [None] * BSL
        xls = [None] * BSL
        xws = [None] * BSL
        xwrs = [None] * BSL
        an1s = [None] * BSL
        ctxT = None

        def alpha_chain(l, s):
            """Masked softmax over incident hyperedges -> alpha (for msg) and
            a2b = alpha * 0.25/D(v) (for the node-side aggregation).
            GpSimd builds the logits, Act does the exp, DVE the reductions."""
            an_src = an0s[s] if l == 0 else an1s[s]
            # logits are O(8) so the softmax runs unshifted; the -50 mask keeps
            # non-incident entries at exp() ~ 1e-19
            t1 = alph.tile([128, NB, HH, M], f32, tag="t1", bufs=2)
            nc.vector.tensor_tensor(t1[:], _ins0(abs_[l][s][:], NB, 0),
                                    _ins0(an_src[:], M, -1), op=ALU.add)
            nc.vector.scalar_tensor_tensor(t1[:], t1[:], 0.2, t1[:],
                                           op0=ALU.mult, op1=ALU.max)
            nc.vector.tensor_tensor(t1[:], t1[:], _ins0(hbs[s][:], HH, 1), op=ALU.add)
            t1e = alph.tile([128, NB, HH, M], bf16, tag="t1e", bufs=2)
            nc.scalar.activation(t1e[:], t1[:], AF.Exp)
            ssum = alph.tile([128, NB, HH], f32, tag="ssum")
            nc.vector.tensor_reduce(ssum[:], t1e[:], axis=AX.X, op=ALU.add)
            rs = alph.tile([128, NB, HH], f32, tag="rs")
            nc.vector.reciprocal(rs[:], ssum[:])
            rcol2 = alph.tile([128, NB, HH], f32, tag="rcol2")
            nc.vector.tensor_tensor(rcol2[:], rs[:], _ins0(ivdqs[s][:], HH, -1), op=ALU.mult)
            al = alph.tile([128, NB, HH, M], bf16, tag="alpha")
            nc.gpsimd.tensor_tensor(al[:], t1e[:], _ins0(rs[:], M, -1), op=ALU.mult)
            a2 = alph.tile([128, NB, HH, M], bf16, tag="a2b")
            nc.gpsimd.tensor_tensor(a2[:], t1e[:], _ins0(rcol2[:], M, -1), op=ALU.mult)
            alphas[s] = al
            a2bs[s] = a2

        def bulk(l, s):
            """x @ [w_lin | w_rel0 | w_rel1 | w_root]: all alpha-independent
            PE work for (l, s), drained chunk-by-chunk into bf16 SBUF."""
            xl = acts.tile([128, HH, NB, C], bf16, tag="xl")
            xw = acts.tile([128, NB, 2, C], bf16, tag="xw")
            xwr = acts.tile([128, NB, C], bf16, tag="xwr", bufs=2)
            for k in range(7):
                for nt in range(NB):
                    if k < HH:
                        stat = xhTs[s]
                        dst = xl[:, k, nt, :]
                    else:
                        stat = xrTs[s]
                        dst = xw[:, nt, k - 4, :] if k < 6 else xwr[:, nt, :]
                    ps = blk.tile([128, C], f32, tag="blk")
                    for ct in range(CT):
                        nc.tensor.matmul(ps[:], stat[:, ct, nt * 128 : (nt + 1) * 128],
                                         ws[l][:, k, ct, :], start=(ct == 0), stop=(ct == CT - 1))
                    copy(dst, ps[:])
            xls[s], xws[s], xwrs[s] = xl, xw, xwr

        def phase2(l, s):
            """alpha-dependent per-sample work: alpha transposes, msg,
            hypergraph out, RGCN aggregation, ctx column."""
            al, a2 = alphas[s], a2bs[s]
            xl, xw, xwr = xls[s], xws[s], xwrs[s]
            Afn = Afns[s]

            a3T = msgs.tile([M, HH, N], bf16, tag="a3T")
            for nb in range(NB):
                for h in range(HH):
                    tp = pmx.tile([M, 128], bf16, tag="mx", padded_shape=[M, 1024])
                    nc.tensor.transpose(tp[:], a2[:, nb, h, :], identb[:])
                    copy(a3T[:, h, nb * 128 : (nb + 1) * 128], tp[:])

            msg = msgs.tile([M, HH, C], bf16, tag="msg")
            for h in range(HH):
                mp = blk.tile([M, C], f32, tag="blk")
                for nb in range(NB):
                    nc.tensor.matmul(mp[:], al[:, nb, h, :], xl[:, h, nb, :],
                                     start=(nb == 0), stop=(nb == NB - 1))
                copy(msg[:, h, :], mp[:], scale=ivbs[s][:, 0:1])

            if l == 0:
                # transposed outputs -> next-layer state tiles [C, N]
                xhT1 = xst.tile([128, CT, N], bf16, tag="xst")
                xrT1 = xst.tile([128, CT, N], bf16, tag="xst")
                for g in range(CT // 2):
                    ph = pst.tile([128, 2, N], f32, tag="st")
                    for j in range(2):
                        ct = 2 * g + j
                        for h in range(HH):
                            nc.tensor.matmul(ph[:, j, :], msg[:, h, ct * 128 : (ct + 1) * 128],
                                             a3T[:, h, :], start=(h == 0), stop=(h == HH - 1))
                    for j in range(2):
                        ct = 2 * g + j
                        relu_bias(xhT1[:, ct, :], ph[:, j, :], bcolh[l][:, ct : ct + 1])
                for g in range(CT // 2):
                    pr = pst.tile([128, 2, N], f32, tag="st")
                    for j in range(2):
                        dt = 2 * g + j
                        first = True
                        for r in range(2):
                            for it in range(NB):
                                nc.tensor.matmul(pr[:, j, :],
                                                 xw[:, it, r, dt * 128 : (dt + 1) * 128],
                                                 Afn[:, r, it, :], start=first, stop=False)
                                first = False
                        for it in range(NB):
                            nc.tensor.matmul(pr[:, j, it * 128 : (it + 1) * 128],
                                             xwr[:, it, dt * 128 : (dt + 1) * 128],
                                             identb[:], start=False, stop=(it == NB - 1))
                    for j in range(2):
                        dt = 2 * g + j
                        relu_bias(xrT1[:, dt, :], pr[:, j, :], bcolr[l][:, dt : dt + 1])
                xhTs[s] = xhT1
                xrTs[s] = xrT1
                # ctx columns (pre-patch node-0 output)
                nc.vector.tensor_copy(ctxT[:, 0:CT, s : s + 1], xrT1[:, :, 0:1])
                nc.vector.tensor_copy(ctxT[:, CT : 2 * CT, s : s + 1], xhT1[:, :, 0:1])
            else:
                # row-major bf16 outputs, streamed to DRAM (rows 1..N-1)
                outh_t = outs.tile([128, NB, C], bf16, tag="outh")
                for nb in range(NB):
                    po = blk.tile([128, C], f32, tag="blk")
                    for h in range(HH):
                        nc.tensor.matmul(po[:], a3T[:, h, nb * 128 : (nb + 1) * 128],
                                         msg[:, h, :], start=(h == 0), stop=False)
                    nc.tensor.matmul(po[:], ones_row[:], browh[l][:], start=False, stop=True)
                    e = cpeng()
                    if e is nc.scalar:
                        e.activation(outh_t[:, nb, :], po[:], AF.Relu)
                    else:
                        e.tensor_scalar(outh_t[:, nb, :], po[:], 0.0, None, op0=ALU.max)
                outr_t = outs.tile([128, NB, C], bf16, tag="outr")
                for jb in range(NB):
                    po = blk.tile([128, C], f32, tag="blk")
                    first = True
                    for r in range(2):
                        for it in range(NB):
                            nc.tensor.matmul(po[:], Afn[:, r, it, jb * 128 : (jb + 1) * 128],
                                             xw[:, it, r, :], start=first, stop=False)
                            first = False
                    nc.tensor.matmul(po[:], identb[:], xwr[:, jb, :], start=False, stop=False)
                    nc.tensor.matmul(po[:], ones_row[:], browr[l][:], start=False, stop=True)
                    e = cpeng()
                    if e is nc.scalar:
                        e.activation(outr_t[:, jb, :], po[:], AF.Relu)
                    else:
                        e.tensor_scalar(outr_t[:, jb, :], po[:], 0.0, None, op0=ALU.max)
                # ctx rows -> columns
                cps = pmx.tile([128, 2, CT, 2], bf16, tag="mx", padded_shape=[128, 2, CT, 64])
                for ct in range(CT):
                    nc.tensor.transpose(cps[:, 0, ct, 0:1],
                                        outr_t[0:1, 0, ct * 128 : (ct + 1) * 128],
                                        identb[0:1, 0:1])
                    nc.tensor.transpose(cps[:, 1, ct, 0:1],
                                        outh_t[0:1, 0, ct * 128 : (ct + 1) * 128],
                                        identb[0:1, 0:1])
                nc.vector.tensor_copy(
                    ctxT[:, :, s : s + 1].rearrange("p (r ct) o -> p r (ct o)", r=2),
                    cps[:, :, :, 0])
                for tsrc, dram in ((outr_t, d_outr), (outh_t, d_outh)):
                    nc.sync.dma_start(dram[s, 1:128, :], tsrc[1:128, 0, :])
                    nc.sync.dma_start(dram[s, 128:N, :], tsrc[:, 1, :])

        def ie(l):
            """info-exchange MLP over the BSL ctx columns; layer 0 patches the
            state tiles' node-0 column, layer 1 DMAs the node-0 output rows."""
            y1 = iep.tile([BSL, C2], bf16, tag="y1")
            for ch in range(2):
                yp = blk.tile([BSL, C], f32, tag="blk")
                for kt in range(KT2):
                    nc.tensor.matmul(yp[:], ctxT[:, kt, :], iw1_t[:, kt, ch * C : (ch + 1) * C],
                                     start=(kt == 0), stop=False)
                nc.tensor.matmul(yp[:], ones4[:], ib1r[l][:, ch * C : (ch + 1) * C],
                                 start=False, stop=True)
                e = cpeng()
                if e is nc.scalar:
                    e.activation(y1[:, ch * C : (ch + 1) * C], yp[:], AF.Relu)
                else:
                    e.tensor_scalar(y1[:, ch * C : (ch + 1) * C], yp[:], 0.0, None, op0=ALU.max)
            c2T = iep.tile([128, KT2, BSL], bf16, tag="c2T")
            for kt in range(KT2):
                tp = pmx.tile([128, BSL], bf16, tag="mx", padded_shape=[128, 1024])
                nc.tensor.transpose(tp[:], y1[:, kt * 128 : (kt + 1) * 128], identb[0:BSL, 0:BSL])
                copy(c2T[:, kt, :], tp[:])
            y2 = iep.tile([BSL, C2], bf16, tag="y2")
            for ch in range(2):
                yp = blk.tile([BSL, C], f32, tag="blk")
                for kt in range(KT2):
                    nc.tensor.matmul(yp[:], c2T[:, kt, :], iw2_t[:, kt, ch * C : (ch + 1) * C],
                                     start=(kt == 0), stop=False)
                nc.tensor.matmul(yp[:], ones4[:], ib2r[l][:, ch * C : (ch + 1) * C],
                                 start=False, stop=True)
                copy(y2[:, ch * C : (ch + 1) * C], yp[:])
            if l == 0:
                for kt in range(KT2):
                    tp = pmx.tile([128, BSL], bf16, tag="mx", padded_shape=[128, 1024])
                    nc.tensor.transpose(tp[:], y2[:, kt * 128 : (kt + 1) * 128],
                                        identb[0:BSL, 0:BSL])
                    for s in range(BSL):
                        dst = xrTs[s] if kt < CT else xhTs[s]
                        e = cpeng()
                        if e is nc.scalar:
                            e.copy(dst[:, kt % CT, 0:1], tp[:, s : s + 1])
                        else:
                            e.tensor_copy(dst[:, kt % CT, 0:1], tp[:, s : s + 1])
            else:
                for s in range(BSL):
                    nc.sync.dma_start(d_outr[s, 0:1, :], y2[s : s + 1, 0:C])
                    nc.sync.dma_start(d_outh[s, 0:1, :], y2[s : s + 1, C:C2])

        # ================= layer 0 =================
        ctxT = iep.tile([128, 2 * CT, BSL], bf16, tag="ctxT")
        for s in range(BSL):
            alpha_chain(0, s)
        # layer-1 / IE weights stream behind the critical path: their issue
        # instructions sit after the alpha-chain work on the gpsimd/vector
        # queues, so the transfers start only once layer 0 is underway
        ws[1] = wts.tile([128, 7, CT, 512], bf16, tag="w", name="w1")
        for k in range(7):
            nc.gpsimd.dma_start(ws[1][:, k, :, :], d_w[1, :, k, :, :])
        iw1_t = wie.tile([128, KT2, C2], bf16, tag="iw1")
        nc.scalar.dma_start(iw1_t[:], d_iw1[0])
        iw2_t = wie.tile([128, KT2, C2], bf16, tag="iw2")
        nc.scalar.dma_start(iw2_t[:], d_iw2[0])
        bulk(0, 0)
        bulk(0, 1)
        phase2(0, 0)
        bulk(0, 2)
        phase2(0, 1)
        bulk(0, 3)
        phase2(0, 2)
        phase2(0, 3)
        ie(0)

        # layer-1 IE weights reuse the same SBUF slots (gated on ie(0) readers)
        iw1_t = wie.tile([128, KT2, C2], bf16, tag="iw1")
        nc.gpsimd.dma_start(iw1_t[:], d_iw1[1])
        iw2_t = wie.tile([128, KT2, C2], bf16, tag="iw2")
        nc.sync.dma_start(iw2_t[:], d_iw2[1])

        # ================= layer 1 =================
        ctxT = iep.tile([128, 2 * CT, BSL], bf16, tag="ctxT2")
        for s in range(BSL):
            ap = blk.tile([HH, N], f32, tag="blk", padded_shape=[HH, 512])
            for ct in range(CT):
                nc.tensor.matmul(ap[:], ux1[:, ct, :], xhTs[s][:, ct, :],
                                 start=(ct == 0), stop=(ct == CT - 1))
            anrow = iep.tile([HH, N], bf16, tag="anrow", name=f"anrow_{s}")
            copy(anrow[:], ap[:])
            an1 = alph.tile([128, NB, HH], f32, tag="an1")
            for nb in range(NB):
                tp = pmx.tile([128, HH], bf16, tag="mx", padded_shape=[128, 1024])
                nc.tensor.transpose(tp[:], anrow[:, nb * 128 : (nb + 1) * 128],
                                    identb[0:HH, 0:HH])
                copy(an1[:, nb, :], tp[:])
            an1s[s] = an1
        for s in range(BSL):
            alpha_chain(1, s)
        bulk(1, 0)
        bulk(1, 1)
        phase2(1, 0)
        bulk(1, 2)
        phase2(1, 1)
        bulk(1, 3)
        phase2(1, 2)
        phase2(1, 3)
        ie(1)

    nc.compile()
    return nc


_NC = None


def _get_nc():
    global _NC
    if _NC is None:
        _NC = build_module()
    return _NC


def make_in_maps(encoded_spans, SVO_emb, pooled_output, sent2word_adj, aug_adj,
                 punct_graph, w_rel, w_root, b_rgcn, w_lin, att_x, att_e, b_hgcn,
                 ie_w1, ie_b1, ie_w2, ie_b2):
    f = np.float32
    bf = ml_dtypes.bfloat16
    x = np.asarray(encoded_spans, f)                               # [BS, N, C]
    aug = np.asarray(aug_adj, f)
    pun = np.asarray(punct_graph, f)
    A = np.stack([pun * (1.0 - aug), aug], axis=1)                 # [BS, 2, N, N]
    deg = A.sum(axis=2)                                            # in-degree of target j
    adjn = A / np.where(deg > 0, deg, 1.0)[:, :, None, :]
    Hinc = np.concatenate([np.ones((BS, N, 1), f),
                           np.asarray(sent2word_adj, f)], axis=2)  # [BS, N, M]
    hbm = np.where(Hinc > 0, 0.0, -50.0).astype(f)
    Dn = Hinc.sum(axis=2)
    ivdq = (0.25 / np.where(Dn > 0, Dn, 1.0)).astype(f)            # [BS, N]
    Be = Hinc.sum(axis=1)
    ivb = np.where(Be > 0, 1.0 / np.where(Be > 0, Be, 1.0), 0.0).astype(f)  # [BS, M]
    e_attr = np.concatenate([np.asarray(pooled_output, f)[:, None, :],
                             np.asarray(SVO_emb, f)], axis=1)      # [BS, M, C]
    wl = np.ascontiguousarray(np.asarray(w_lin, f))                # [L, C, HH*C]
    wl4 = wl.reshape(L, C, HH, C)
    ux = np.einsum("lchk,lhk->lch", wl4, np.asarray(att_x, f))     # [L, C, HH]
    ue = np.einsum("lchk,lhk->lch", wl4, np.asarray(att_e, f))
    an0 = np.einsum("bnc,ch->bnh", x, ux[0]).astype(f)             # [BS, N, HH]
    ae = np.einsum("bmc,lch->lbhm", e_attr, ue)                    # [L, BS, HH, M]
    wr = np.asarray(w_rel, f)
    wcat = np.concatenate([wr[:, 0], wr[:, 1], np.asarray(w_root, f)], axis=2)
    x0T = np.ascontiguousarray(x.transpose(0, 2, 1))               # [BS, C, N]

    # partition-major swizzles: index (ct*128+p) -> [p][ct]
    wcomb = np.concatenate([wl, wcat], axis=2)                     # [L, C, 3584]
    wcomb = wcomb.reshape(L, CT, 128, 7, 512).transpose(0, 2, 3, 1, 4)  # [L,128,7,CT,512]
    x0Tp = x0T.reshape(BS, CT, 128, N).transpose(0, 2, 1, 3)       # [BS,128,CT,N]
    adjnp = adjn.reshape(BS, 2, NB, 128, N).transpose(0, 3, 1, 2, 4)  # [BS,128,2,NB,N]
    hbp = hbm.reshape(BS, NB, 128, M).transpose(0, 2, 1, 3)        # [BS,128,NB,M]
    an0p = an0.reshape(BS, NB, 128, HH).transpose(0, 2, 1, 3)      # [BS,128,NB,HH]
    ivdqp = ivdq.reshape(BS, NB, 128).transpose(0, 2, 1)           # [BS,128,NB]
    iw1p = np.asarray(ie_w1, f).reshape(L, KT2, 128, C2).transpose(0, 2, 1, 3)
    iw2p = np.asarray(ie_w2, f).reshape(L, KT2, 128, C2).transpose(0, 2, 1, 3)
    ux1p = ux[1].reshape(CT, 128, HH).transpose(1, 0, 2)           # [128,CT,HH]
    bcolrp = np.asarray(b_rgcn, f).reshape(L, CT, 128).transpose(0, 2, 1)
    bcolhp = np.asarray(b_hgcn, f).reshape(L, CT, 128).transpose(0, 2, 1)

    shared = {
        "wcomb": np.ascontiguousarray(wcomb).astype(bf),
        "ux1": np.ascontiguousarray(ux1p).astype(bf),
        "iw1": np.ascontiguousarray(iw1p).astype(bf),
        "iw2": np.ascontiguousarray(iw2p).astype(bf),
        "bcolr": np.ascontiguousarray(bcolrp),
        "bcolh": np.ascontiguousarray(bcolhp),
        "browr": np.asarray(b_rgcn, f).astype(bf),
        "browh": np.asarray(b_hgcn, f).astype(bf),
        "ib1": np.asarray(ie_b1, f).astype(bf),
        "ib2": np.asarray(ie_b2, f).astype(bf),
        "eyeb": np.eye(128, dtype=f).astype(bf),
        "onesr": np.ones((1, 128), f).astype(bf),
        "ones4": np.ones((1, 4), f).astype(bf),
    }
    in_maps = []
    for c in range(NCORES):
        sl = slice(c * BSL, (c + 1) * BSL)
        m = dict(shared)
        m["x0T"] = np.ascontiguousarray(x0Tp[sl]).astype(bf)
        m["adjn"] = np.ascontiguousarray(adjnp[sl]).astype(bf)
        m["hbm"] = np.ascontiguousarray(hbp[sl]).astype(bf)
        m["ivdq"] = np.ascontiguousarray(ivdqp[sl])
        m["ivb"] = np.ascontiguousarray(ivb[sl])[:, :, None]
        m["an0"] = np.ascontiguousarray(an0p[sl])
        m["ae"] = np.ascontiguousarray(ae[:, sl]).astype(bf)
        in_maps.append(m)
    return in_maps


def run(in_maps, trace=False, **kw):
    nc = _get_nc()
    return run_bass_kernel_spmd(nc, in_maps, list(range(NCORES)), trace=trace, **kw)


def kernel(**inputs):
    in_maps = make_in_maps(**inputs)
    res = run(in_maps)
    x_r = np.concatenate([np.asarray(res.results[c]["outr"]) for c in range(NCORES)], axis=0)
    x_h = np.concatenate([np.asarray(res.results[c]["outh"]) for c in range(NCORES)], axis=0)
    return x_r.astype(np.float32), x_h.astype(np.float32)
